# revision 21
# baseline (speedup 1.0000x reference)
"""EnhancedVLAD Trainium2 kernel — pure data-parallel over 8 NeuronCores.

Math (validated against the reference):
  xn = x / max(||x||_c, eps)
  assign = softmax_k(conv_w @ xn + conv_b)          (logits bounded, no max-sub needed)
  agg[k,c] = sum_n assign[k,n] * xn[c,n] ;  mass[k] = sum_n assign[k,n]
  vlad = agg - centroids * mass[:,None]
  The ghost down-weighting and the attention row-scales are strictly positive
  per-row scalars, so they cancel in the per-row L2 normalization; ghost rows
  are dropped. Each kept row is unit-norm, so the global L2 norm is exactly
  sqrt(64) = 8.  =>  out = rownorm(vlad[:64]) / 8.

Device pipeline per core (B_local=4 batches, processed as 8 half-batch units
of 2048 columns for pipeline depth):
  x_nat  [128c_p, 4q, 2048n] bf16    (one SWDGE cast-load per unit)
  xT     [128n_p, 16t, 4q, 128c] bf16  (xbar DMA transpose, SP queue)
  stage1: logitsT[128n,72] = sum_q x_nat_q^T @ conv_wT_q        (PE, bf16)
  norms:  ||x_col||^2 split: even tiles ACT Square+accum_out,
          odd tiles POOL square (tensor_mul) + DVE reduce
  softmax: ACT Exp(scale=1/||x||); sumexp via one DVE group-reduce
  stage2: agg[64,512] += s_t^T @ xT_t ; mass[64,1] += s_t^T @ norm_col  (PE)
  epilogue: vlad = agg - cent*mass; out = vlad * (1/max(||row||,eps)) / 8

Sync notes: TRN2 instructions carry at most ONE semaphore wait; the module is
built as bacc.Bacc and nc.compile() legalizes multi-wait instructions into
event-semaphore chains.  SP-queue event-sem waits are expensive (~2us), so
tiny ACT-queue absorber copies carry the transposes' WAR/WAW waits and each
XPOSE is left with only its load wait.  DVE tensor_tensor_reduce is avoided
(compiles + simulates but crashes real HW).
"""

import os
import sys

for _p in ("/opt/trn_rl_repo", "/opt/pypackages"):
    if _p not in sys.path and os.path.isdir(_p):
        sys.path.insert(0, _p)

import numpy as np
import ml_dtypes

import concourse.bass as bass
import concourse.bacc as bacc
import concourse.mybir as mybir
from concourse import tile
from concourse.bass_utils import run_bass_kernel_spmd

F32 = mybir.dt.float32
BF16 = mybir.dt.bfloat16
AF = mybir.ActivationFunctionType
OP = mybir.AluOpType

N_CORES = 8
B_TOTAL, C, N = 32, 512, 4096
B_LOC = B_TOTAL // N_CORES          # 4
T_CL, K_CL = 72, 64                 # clusters (with ghosts), kept clusters
NQ = C // 128                       # 4 c-chunks
N_H = N // 2                        # half-batch columns
NT_H = N_H // 128                   # 16 n-tiles per unit
NT = N // 128                       # 32 n-tiles per batch
GRP = 4                             # n-tiles per PSUM logits group
EPS = 1e-12


def _build_program(with_bias: bool) -> bass.Bass:
    nc = bacc.Bacc("TRN2", target_bir_lowering=False, debug=False)

    x_d = nc.declare_dram_parameter("x", [B_LOC, C, N], F32, isOutput=False)
    cwt_d = nc.declare_dram_parameter("convwt", [128, NQ, T_CL], BF16, isOutput=False)
    cent_d = nc.declare_dram_parameter("cent", [K_CL, C], F32, isOutput=False)
    if with_bias:
        cb_d = nc.declare_dram_parameter("convb", [1, T_CL], BF16, isOutput=False)
    out_d = nc.declare_dram_parameter("out", [B_LOC, K_CL * C], F32, isOutput=True)

    with tile.TileContext(nc) as tc:
        with (
            tc.tile_pool(name="const", bufs=1) as constp,
            tc.tile_pool(name="xnat", bufs=4) as xnatp,
            tc.tile_pool(name="xt", bufs=4) as xtp,
            tc.tile_pool(name="strip", bufs=4) as stripp,
            tc.tile_pool(name="gstrip", bufs=6) as gstripp,
            tc.tile_pool(name="scr", bufs=4) as scrp,
            tc.tile_pool(name="ex", bufs=3) as exp_pool,
            tc.tile_pool(name="sg", bufs=3) as sgp,
            tc.tile_pool(name="epi", bufs=2) as epip,
            tc.tile_pool(name="lg", bufs=3, space="PSUM") as lgp,
            tc.tile_pool(name="agg", bufs=2, space="PSUM") as aggp,
            tc.tile_pool(name="mass", bufs=2, space="PSUM") as massp,
        ):
            cwt = constp.tile([128, NQ, T_CL], BF16)
            nc.sync.dma_start(cwt[:], cwt_d[:])
            cent = constp.tile([K_CL, C], F32)
            nc.sync.dma_start(cent[:], cent_d[:])
            if with_bias:
                ones_row = constp.tile([1, 128], BF16)
                nc.vector.memset(ones_row[:], 1.0)
                cb = constp.tile([1, T_CL], BF16)
                nc.sync.dma_start(cb[:], cb_d[:])

            tr = constp.tile([1, 4], F32)
            last_lg = None
            last_sc = None
            xT_hist: dict = {}
            agg = mass = None

            for u in range(2 * B_LOC):
                b, h = divmod(u, 2)
                # ---- load + cast fp32 -> bf16 (SWDGE), one 4MB DMA ----
                x_nat = xnatp.tile([128, NQ, N_H], BF16, tag="xnat")
                nc.gpsimd.dma_start(
                    x_nat[:],
                    x_d[b, :, h * N_H : (h + 1) * N_H].rearrange(
                        "(q p) n -> p q n", p=128
                    ),
                )

                # ---- transpose xT[p,t,q,f] = x[c=128q+f, n=128t+p] ----
                absorbers = []
                if u - 4 in xT_hist:
                    absorbers.append(nc.scalar.copy(tr[0:1, 0:2], last_lg[0:1, 0:1, 0:2]))
                    absorbers.append(nc.scalar.copy(tr[0:1, 0:2], last_sc[0:1, 0:2]))
                    absorbers.append(
                        nc.scalar.copy(tr[0:1, 0:2], xT_hist[u - 4][0:1, 0:1, 0:1, 0:2]))
                if u - 1 in xT_hist:
                    absorbers.append(
                        nc.scalar.copy(tr[0:1, 0:2], xT_hist[u - 1][0:1, 0:1, 0:1, 0:2]))
                absorbers.append(nc.scalar.copy(tr[0:1, 0:2], x_nat[0:1, 0:1, 0:2]))
                xT = xtp.tile([128, NT_H, NQ, 128], BF16, tag="xt")
                xT_hist[u] = xT
                for q in range(NQ):
                    tp = nc.sync.dma_start_transpose(xT[:, :, q, :], x_nat[:, q, :])
                    for a in absorbers:
                        bass._add_dep_helper(tp.ins, a.ins, sync=False,
                                             reason="xpose after wait-absorbers")

                # ---- per-column channel norms (split ACT / POOL+DVE) ----
                n2 = stripp.tile([128, NT_H], F32, tag="n2")
                for t in range(NT_H):
                    if t % 2 == 0:
                        scr = scrp.tile([128, NQ, 128], BF16, tag="scrA")
                        nc.scalar.activation(
                            scr[:], xT[:, t, :, :], AF.Square,
                            accum_out=n2[:, t : t + 1],
                        )
                    else:
                        scr = scrp.tile([128, NQ, 128], BF16, tag="scrP")
                        nc.gpsimd.tensor_mul(scr[:], xT[:, t, :, :], xT[:, t, :, :])
                        nc.vector.tensor_reduce(
                            n2[:, t : t + 1], scr[:], mybir.AxisListType.XY, OP.add
                        )

                nrm = stripp.tile([128, NT_H], F32, tag="nrm")
                nc.scalar.activation(nrm[:], n2[:], AF.Sqrt)
                nc.vector.tensor_scalar_max(nrm[:], nrm[:], EPS)
                inv = stripp.tile([128, NT_H], F32, tag="inv")
                nc.vector.reciprocal(inv[:], nrm[:])
                nrmb = stripp.tile([128, NT_H], BF16, tag="nrmb")
                nc.vector.tensor_copy(nrmb[:], nrm[:])

                if h == 0:
                    agg = aggp.tile([K_CL, C], F32, tag="agg")
                    mass = massp.tile([K_CL, 1], F32, tag="mass")

                for g in range(NT_H // GRP):
                    # ---- stage 1 ----
                    lg = lgp.tile([128, GRP, T_CL], F32, tag="lg")
                    for i in range(GRP):
                        t = g * GRP + i
                        for q in range(NQ):
                            nc.tensor.matmul(
                                lg[:, i, :],
                                x_nat[:, q, bass.ts(t, 128)],
                                cwt[:, q, :],
                                start=(q == 0),
                                stop=(q == NQ - 1) if not with_bias else False,
                            )
                        if with_bias:
                            nc.tensor.matmul(
                                lg[:, i, :], ones_row[:], cb[:],
                                start=False, stop=True,
                            )

                    # ---- softmax ----
                    se = gstripp.tile([128, GRP], F32, tag="se")
                    ex = exp_pool.tile([128, GRP, T_CL], F32, tag="ex")
                    for i in range(GRP):
                        t = g * GRP + i
                        nc.scalar.activation(
                            ex[:, i, :], lg[:, i, :], AF.Exp,
                            scale=inv[:, t : t + 1],
                        )
                    nc.vector.tensor_reduce(se[:], ex[:], mybir.AxisListType.X, OP.add)
                    sc = gstripp.tile([128, GRP], F32, tag="sc")
                    nc.vector.reciprocal(sc[:], se[:])
                    nc.vector.tensor_mul(
                        sc[:], sc[:], inv[:, g * GRP : (g + 1) * GRP]
                    )
                    last_lg, last_sc = lg, sc

                    sg = sgp.tile([128, GRP, K_CL], BF16, tag="sg")
                    for i in range(GRP):
                        t = g * GRP + i
                        tt = h * NT_H + t
                        nc.vector.tensor_scalar(
                            sg[:, i, :], ex[:, i, 0:K_CL],
                            sc[:, i : i + 1], None, OP.mult,
                        )
                        # ---- stage 2 ----
                        nc.tensor.matmul(
                            agg[:], sg[:, i, :], xT[:, t, :, :],
                            start=(tt == 0), stop=(tt == NT - 1),
                        )
                        nc.tensor.matmul(
                            mass[:], sg[:, i, :], nrmb[:, t : t + 1],
                            start=(tt == 0), stop=(tt == NT - 1),
                        )

                if h == 1:
                    # ---- epilogue ----
                    mass_sb = epip.tile([K_CL, 1], F32, tag="mass_sb")
                    nc.vector.tensor_copy(mass_sb[:], mass[:])
                    cm = epip.tile([K_CL, C], F32, tag="cm")
                    nc.vector.tensor_scalar(cm[:], cent[:], mass_sb[:], None, OP.mult)
                    vlad = epip.tile([K_CL, C], F32, tag="vlad")
                    nc.vector.tensor_sub(vlad[:], agg[:], cm[:])

                    vsq = epip.tile([K_CL, C], BF16, tag="vsq")
                    rn2 = epip.tile([K_CL, 1], F32, tag="rn2")
                    nc.scalar.activation(vsq[:], vlad[:], AF.Square, accum_out=rn2[:])
                    rn = epip.tile([K_CL, 1], F32, tag="rn")
                    nc.scalar.activation(rn[:], rn2[:], AF.Sqrt)
                    nc.vector.tensor_scalar_max(rn[:], rn[:], EPS)
                    rinv = epip.tile([K_CL, 1], F32, tag="rinv")
                    nc.vector.reciprocal(rinv[:], rn[:])

                    ob = epip.tile([K_CL, C], F32, tag="ob")
                    nc.vector.tensor_scalar(
                        ob[:], vlad[:], rinv[:], 0.125, OP.mult, OP.mult
                    )
                    nc.gpsimd.dma_start(
                        out_d[b].rearrange("(k c) -> k c", c=C), ob[:]
                    )

    nc.compile()
    return nc


_CACHE: dict = {}


def _get_program(with_bias: bool) -> bass.Bass:
    key = ("prog", with_bias)
    if key not in _CACHE:
        _CACHE[key] = _build_program(with_bias)
    return _CACHE[key]


def _prep_params(conv_w: np.ndarray, centroids: np.ndarray):
    # conv_wT chunked: convwt[p, q, k] = conv_w[k, 128q + p]
    cwt = np.ascontiguousarray(
        conv_w.T.reshape(NQ, 128, T_CL).transpose(1, 0, 2)
    ).astype(ml_dtypes.bfloat16)
    cent = np.ascontiguousarray(centroids[:K_CL]).astype(np.float32)
    return cwt, cent


def kernel(x, centroids, conv_w, conv_b, ghost_weights, w1, b1, w2, b2) -> np.ndarray:
    x = np.asarray(x, dtype=np.float32)
    with_bias = bool(np.any(np.asarray(conv_b)))
    nc = _get_program(with_bias)
    cwt, cent = _prep_params(np.asarray(conv_w, np.float32),
                             np.asarray(centroids, np.float32))

    in_maps = []
    for i in range(N_CORES):
        m = {
            "x": np.ascontiguousarray(x[i * B_LOC : (i + 1) * B_LOC]),
            "convwt": cwt,
            "cent": cent,
        }
        if with_bias:
            m["convb"] = np.asarray(conv_b, np.float32).reshape(1, T_CL).astype(
                ml_dtypes.bfloat16
            )
        in_maps.append(m)

    res = run_bass_kernel_spmd(nc, in_maps, core_ids=list(range(N_CORES)))
    out = np.concatenate([r["out"] for r in res.results], axis=0)
    return np.ascontiguousarray(out.astype(np.float32))


# revision 22
# speedup vs baseline: 1.0573x; 1.0573x over previous
"""EnhancedVLAD Trainium2 kernel — pure data-parallel over 8 NeuronCores.

Math (validated against the reference):
  xn = x / max(||x||_c, eps)
  assign = softmax_k(conv_w @ xn + conv_b)          (logits bounded, no max-sub needed)
  agg[k,c] = sum_n assign[k,n] * xn[c,n] ;  mass[k] = sum_n assign[k,n]
  vlad = agg - centroids * mass[:,None]
  The ghost down-weighting and the attention row-scales are strictly positive
  per-row scalars, so they cancel in the per-row L2 normalization; ghost rows
  are dropped. Each kept row is unit-norm, so the global L2 norm is exactly
  sqrt(64) = 8.  =>  out = rownorm(vlad[:64]) / 8.

Device pipeline per core (B_local=4 batches, processed as 8 half-batch units
of 2048 columns for pipeline depth):
  x_nat  [128c_p, 4q, 2048n] bf16    (one SWDGE cast-load per unit)
  xT     [128n_p, 16t, 4q, 128c] bf16  (xbar DMA transpose, SP queue)
  stage1: logitsT[128n,72] = sum_q x_nat_q^T @ conv_wT_q        (PE, bf16)
  norms:  ||x_col||^2 split: even tiles ACT Square+accum_out,
          odd tiles POOL square (tensor_mul) + DVE reduce
  softmax: ACT Exp(scale=1/||x||); sumexp via one DVE group-reduce
  stage2: agg[64,512] += s_t^T @ xT_t ; mass[64,1] += s_t^T @ norm_col  (PE)
  epilogue: vlad = agg - cent*mass; out = vlad * (1/max(||row||,eps)) / 8

Sync notes: TRN2 instructions carry at most ONE semaphore wait; the module is
built as bacc.Bacc and nc.compile() legalizes multi-wait instructions into
event-semaphore chains.  SP-queue event-sem waits are expensive (~2us), so
tiny ACT-queue absorber copies carry the transposes' WAR/WAW waits and each
XPOSE is left with only its load wait.  DVE tensor_tensor_reduce is avoided
(compiles + simulates but crashes real HW).
"""

import os
import sys

for _p in ("/opt/trn_rl_repo", "/opt/pypackages"):
    if _p not in sys.path and os.path.isdir(_p):
        sys.path.insert(0, _p)

import numpy as np
import ml_dtypes

import concourse.bass as bass
import concourse.bacc as bacc
import concourse.mybir as mybir
from concourse import tile
from concourse.bass_utils import run_bass_kernel_spmd

F32 = mybir.dt.float32
BF16 = mybir.dt.bfloat16
AF = mybir.ActivationFunctionType
OP = mybir.AluOpType

N_CORES = 8
B_TOTAL, C, N = 32, 512, 4096
B_LOC = B_TOTAL // N_CORES          # 4
T_CL, K_CL = 72, 64                 # clusters (with ghosts), kept clusters
NQ = C // 128                       # 4 c-chunks
N_H = N // 2                        # half-batch columns
NT_H = N_H // 128                   # 16 n-tiles per unit
NT = N // 128                       # 32 n-tiles per batch
GRP = 4                             # n-tiles per PSUM logits group
EPS = 1e-12


def _build_program(with_bias: bool) -> bass.Bass:
    nc = bacc.Bacc("TRN2", target_bir_lowering=False, debug=False)

    x_d = nc.declare_dram_parameter("x", [B_LOC, C, N], F32, isOutput=False)
    cwt_d = nc.declare_dram_parameter("convwt", [128, NQ, T_CL], BF16, isOutput=False)
    cent_d = nc.declare_dram_parameter("cent", [K_CL, C], F32, isOutput=False)
    if with_bias:
        cb_d = nc.declare_dram_parameter("convb", [1, T_CL], BF16, isOutput=False)
    out_d = nc.declare_dram_parameter("out", [B_LOC, K_CL * C], F32, isOutput=True)

    with tile.TileContext(nc) as tc:
        with (
            tc.tile_pool(name="const", bufs=1) as constp,
            tc.tile_pool(name="xnat", bufs=5) as xnatp,
            tc.tile_pool(name="xt", bufs=5) as xtp,
            tc.tile_pool(name="strip", bufs=4) as stripp,
            tc.tile_pool(name="gstrip", bufs=6) as gstripp,
            tc.tile_pool(name="scr", bufs=4) as scrp,
            tc.tile_pool(name="ex", bufs=3) as exp_pool,
            tc.tile_pool(name="sg", bufs=3) as sgp,
            tc.tile_pool(name="epi", bufs=2) as epip,
            tc.tile_pool(name="lg", bufs=3, space="PSUM") as lgp,
            tc.tile_pool(name="agg", bufs=2, space="PSUM") as aggp,
            tc.tile_pool(name="mass", bufs=2, space="PSUM") as massp,
        ):
            cwt = constp.tile([128, NQ, T_CL], BF16)
            nc.sync.dma_start(cwt[:], cwt_d[:])
            cent = constp.tile([K_CL, C], F32)
            nc.sync.dma_start(cent[:], cent_d[:])
            if with_bias:
                ones_row = constp.tile([1, 128], BF16)
                nc.vector.memset(ones_row[:], 1.0)
                cb = constp.tile([1, T_CL], BF16)
                nc.sync.dma_start(cb[:], cb_d[:])

            tr = constp.tile([1, 4], F32)
            lg_first: dict = {}
            sc_first: dict = {}
            xT_hist: dict = {}
            agg = mass = None

            for u in range(2 * B_LOC):
                b, h = divmod(u, 2)
                # ---- load + cast fp32 -> bf16 (SWDGE), one 4MB DMA ----
                x_nat = xnatp.tile([128, NQ, N_H], BF16, tag="xnat")
                nc.gpsimd.dma_start(
                    x_nat[:],
                    x_d[b, :, h * N_H : (h + 1) * N_H].rearrange(
                        "(q p) n -> p q n", p=128
                    ),
                )

                # ---- transpose xT[p,t,q,f] = x[c=128q+f, n=128t+p] ----
                absorbers = []
                if u - 5 in xT_hist:
                    # dominance: PE/DVE ticks of unit u-4's first group are
                    # newer than every reader of xT[u-5]
                    absorbers.append(nc.scalar.copy(tr[0:1, 0:2], lg_first[u - 4][0:1, 0:1, 0:2]))
                    absorbers.append(nc.scalar.copy(tr[0:1, 0:2], sc_first[u - 4][0:1, 0:2]))
                    absorbers.append(
                        nc.scalar.copy(tr[0:1, 0:2], xT_hist[u - 5][0:1, 0:1, 0:1, 0:2]))
                if u - 1 in xT_hist:
                    absorbers.append(
                        nc.scalar.copy(tr[0:1, 0:2], xT_hist[u - 1][0:1, 0:1, 0:1, 0:2]))
                absorbers.append(nc.scalar.copy(tr[0:1, 0:2], x_nat[0:1, 0:1, 0:2]))
                xT = xtp.tile([128, NQ, NT_H, 128], BF16, tag="xt")
                xT_hist[u] = xT
                tp = nc.sync.dma_start_transpose(
                    xT[:].rearrange("p a b c -> p (a b) c"),
                    x_nat[:].rearrange("p q n -> p (q n)"),
                )
                for a in absorbers:
                    bass._add_dep_helper(tp.ins, a.ins, sync=False,
                                         reason="xpose after wait-absorbers")

                # ---- per-column channel norms (split ACT / POOL+DVE) ----
                n2 = stripp.tile([128, NT_H], F32, tag="n2")
                for t in range(NT_H):
                    if t % 2 == 0:
                        scr = scrp.tile([128, NQ, 128], BF16, tag="scrA")
                        nc.scalar.activation(
                            scr[:], xT[:, :, t, :], AF.Square,
                            accum_out=n2[:, t : t + 1],
                        )
                    else:
                        scr = scrp.tile([128, NQ, 128], BF16, tag="scrP")
                        nc.gpsimd.tensor_mul(scr[:], xT[:, :, t, :], xT[:, :, t, :])
                        nc.vector.tensor_reduce(
                            n2[:, t : t + 1], scr[:], mybir.AxisListType.XY, OP.add
                        )

                nrm = stripp.tile([128, NT_H], F32, tag="nrm")
                nc.scalar.activation(nrm[:], n2[:], AF.Sqrt)
                nc.vector.tensor_scalar_max(nrm[:], nrm[:], EPS)
                inv = stripp.tile([128, NT_H], F32, tag="inv")
                nc.vector.reciprocal(inv[:], nrm[:])
                nrmb = stripp.tile([128, NT_H], BF16, tag="nrmb")
                nc.vector.tensor_copy(nrmb[:], nrm[:])

                if h == 0:
                    agg = aggp.tile([K_CL, C], F32, tag="agg")
                    mass = massp.tile([K_CL, 1], F32, tag="mass")

                for g in range(NT_H // GRP):
                    # ---- stage 1 ----
                    lg = lgp.tile([128, GRP, T_CL], F32, tag="lg")
                    for i in range(GRP):
                        t = g * GRP + i
                        for q in range(NQ):
                            nc.tensor.matmul(
                                lg[:, i, :],
                                x_nat[:, q, bass.ts(t, 128)],
                                cwt[:, q, :],
                                start=(q == 0),
                                stop=(q == NQ - 1) if not with_bias else False,
                            )
                        if with_bias:
                            nc.tensor.matmul(
                                lg[:, i, :], ones_row[:], cb[:],
                                start=False, stop=True,
                            )

                    # ---- softmax ----
                    se = gstripp.tile([128, GRP], F32, tag="se")
                    ex = exp_pool.tile([128, GRP, T_CL], F32, tag="ex")
                    for i in range(GRP):
                        t = g * GRP + i
                        nc.scalar.activation(
                            ex[:, i, :], lg[:, i, :], AF.Exp,
                            scale=inv[:, t : t + 1],
                        )
                    nc.vector.tensor_reduce(se[:], ex[:], mybir.AxisListType.X, OP.add)
                    sc = gstripp.tile([128, GRP], F32, tag="sc")
                    nc.vector.reciprocal(sc[:], se[:])
                    nc.vector.tensor_mul(
                        sc[:], sc[:], inv[:, g * GRP : (g + 1) * GRP]
                    )
                    if g == 0:
                        lg_first[u] = lg
                        sc_first[u] = sc

                    sg = sgp.tile([128, GRP, K_CL], BF16, tag="sg")
                    for i in range(GRP):
                        t = g * GRP + i
                        tt = h * NT_H + t
                        nc.vector.tensor_scalar(
                            sg[:, i, :], ex[:, i, 0:K_CL],
                            sc[:, i : i + 1], None, OP.mult,
                        )
                        # ---- stage 2 ----
                        nc.tensor.matmul(
                            agg[:], sg[:, i, :], xT[:, :, t, :],
                            start=(tt == 0), stop=(tt == NT - 1),
                        )
                        nc.tensor.matmul(
                            mass[:], sg[:, i, :], nrmb[:, t : t + 1],
                            start=(tt == 0), stop=(tt == NT - 1),
                        )

                if h == 1:
                    # ---- epilogue ----
                    mass_sb = epip.tile([K_CL, 1], F32, tag="mass_sb")
                    nc.vector.tensor_copy(mass_sb[:], mass[:])
                    cm = epip.tile([K_CL, C], F32, tag="cm")
                    nc.vector.tensor_scalar(cm[:], cent[:], mass_sb[:], None, OP.mult)
                    vlad = epip.tile([K_CL, C], F32, tag="vlad")
                    nc.vector.tensor_sub(vlad[:], agg[:], cm[:])

                    vsq = epip.tile([K_CL, C], BF16, tag="vsq")
                    rn2 = epip.tile([K_CL, 1], F32, tag="rn2")
                    nc.scalar.activation(vsq[:], vlad[:], AF.Square, accum_out=rn2[:])
                    rn = epip.tile([K_CL, 1], F32, tag="rn")
                    nc.scalar.activation(rn[:], rn2[:], AF.Sqrt)
                    nc.vector.tensor_scalar_max(rn[:], rn[:], EPS)
                    rinv = epip.tile([K_CL, 1], F32, tag="rinv")
                    nc.vector.reciprocal(rinv[:], rn[:])

                    ob = epip.tile([K_CL, C], F32, tag="ob")
                    nc.vector.tensor_scalar(
                        ob[:], vlad[:], rinv[:], 0.125, OP.mult, OP.mult
                    )
                    nc.gpsimd.dma_start(
                        out_d[b].rearrange("(k c) -> k c", c=C), ob[:]
                    )

    nc.compile()
    return nc


_CACHE: dict = {}


def _get_program(with_bias: bool) -> bass.Bass:
    key = ("prog", with_bias)
    if key not in _CACHE:
        _CACHE[key] = _build_program(with_bias)
    return _CACHE[key]


def _prep_params(conv_w: np.ndarray, centroids: np.ndarray):
    # conv_wT chunked: convwt[p, q, k] = conv_w[k, 128q + p]
    cwt = np.ascontiguousarray(
        conv_w.T.reshape(NQ, 128, T_CL).transpose(1, 0, 2)
    ).astype(ml_dtypes.bfloat16)
    cent = np.ascontiguousarray(centroids[:K_CL]).astype(np.float32)
    return cwt, cent


def kernel(x, centroids, conv_w, conv_b, ghost_weights, w1, b1, w2, b2) -> np.ndarray:
    x = np.asarray(x, dtype=np.float32)
    with_bias = bool(np.any(np.asarray(conv_b)))
    nc = _get_program(with_bias)
    cwt, cent = _prep_params(np.asarray(conv_w, np.float32),
                             np.asarray(centroids, np.float32))

    in_maps = []
    for i in range(N_CORES):
        m = {
            "x": np.ascontiguousarray(x[i * B_LOC : (i + 1) * B_LOC]),
            "convwt": cwt,
            "cent": cent,
        }
        if with_bias:
            m["convb"] = np.asarray(conv_b, np.float32).reshape(1, T_CL).astype(
                ml_dtypes.bfloat16
            )
        in_maps.append(m)

    res = run_bass_kernel_spmd(nc, in_maps, core_ids=list(range(N_CORES)))
    out = np.concatenate([r["out"] for r in res.results], axis=0)
    return np.ascontiguousarray(out.astype(np.float32))
